# revision 1
# baseline (speedup 1.0000x reference)
"""Trainium2 Bass kernel for AdaptiveMixtureOfExperts (top-2 SwiGLU MoE).

Strategy (expert-parallel with FF-split load balancing):
  - Host computes the tiny router (x @ Wr, top-2, softmax) with jax-on-CPU ops
    that bit-match the reference, then shards tokens by routed expert.
  - Each expert's FFN is split in half along D_FF; each NeuronCore runs two
    half-FFN "sections": section A = FF-half h of one of the 4 largest
    experts, section B = FF-half h of one of the 4 smallest (cores 0-3 take
    h=0, cores 4-7 take h=1).  This balances per-core work to the average of
    a large+small expert instead of the max expert.
        hT = W1h.T @ xgT            (ff on partitions, tokens on free dim)
        uT = (a + b1a) * silu(g + b1g)
        yT_partial = W2h.T @ uT     (+ b2 on half-0 cores only)
  - Host sums the two half contributions per expert, applies the top-2
    combine weights, and scatter-adds into the full [B, S, D] output.

Shapes hardcoded for the problem instance:
  x:[2,2048,1024] f32, Wr:[1024,8], temp:[1], W1:[8,1024,4096], b1:[8,4096],
  W2:[8,2048,1024], b2:[8,1024].  TOP_K=2, 8 experts on 8 cores.
"""

import os

import numpy as np
import ml_dtypes

D_MODEL = 1024
D_FF = 2048
NUM_EXPERTS = 8
TOP_K = 2
P = 128          # partitions
NT = 512         # token tile (moving free dim per matmul)
N_CORES = 8
FH = D_FF // 2   # ff half

_NC_CACHE = {}
LAST_RESULTS = None  # test harness introspection


def _build_nc(CA: int, CB: int, use_silu: bool = True):
    """Per-core Bass graph: two half-FF FFN sections of CA and CB tokens.

    use_silu=False decomposes silu into sigmoid+mul (CoreSim has no Silu).
    """
    import concourse.mybir as mybir
    import concourse.tile as tile
    from concourse import bacc
    from concourse.bass import ts

    f32 = mybir.dt.float32
    bf16 = mybir.dt.bfloat16
    AF = mybir.ActivationFunctionType

    K1 = D_MODEL // P          # 8 k-tiles for matmul1
    K2 = FH // P               # 8 k-tiles for matmul2 (half ff)
    NF1 = 2 * FH // P          # 16 ff tiles of hT per section (a+g)
    NO = D_MODEL // P          # 8 out tiles of yT
    WCHUNK = 512

    def tile_bounds(C):
        n_t = (C + NT - 1) // NT
        cuts = [min(NT * i, C) for i in range(n_t)] + [C]
        return [(cuts[i], cuts[i + 1] - cuts[i]) for i in range(n_t)]

    nc = bacc.Bacc()
    xT = {}
    w1 = {}
    w2 = {}
    b1t = {}
    outp = {}
    secs = [("a", CA), ("b", CB)]
    for s, C in secs:
        xT[s] = nc.declare_dram_parameter(f"x{s}T", [D_MODEL, C], bf16, isOutput=False)
        w1[s] = nc.declare_dram_parameter(f"w1{s}", [D_MODEL, 2 * FH], bf16, isOutput=False)
        w2[s] = nc.declare_dram_parameter(f"w2{s}", [FH, D_MODEL], bf16, isOutput=False)
        b1t[s] = nc.declare_dram_parameter(f"b1t{s}", [P, NF1], f32, isOutput=False)
        # partial y without b2 (host adds the bias once per expert), bf16 to
        # halve output DMA bytes
        outp[s] = nc.declare_dram_parameter(f"out{s}", [D_MODEL, C], bf16, isOutput=True)

    with tile.TileContext(nc) as tc:
        with (
            tc.tile_pool(name="weights", bufs=1) as wpool,
            tc.tile_pool(name="acts", bufs=2) as upool,
            tc.tile_pool(name="epilogue", bufs=4) as epool,
            tc.tile_pool(name="ps", bufs=8, space="PSUM") as ps_pool,
        ):
            # ---- small early inputs on qACT (biases) ----
            b1_sb = {}
            for s, C in secs:
                b1_sb[s] = wpool.tile([P, NF1], f32, name=f"b1_sb{s}", tag=f"b1{s}")
                nc.scalar.dma_start(out=b1_sb[s][:], in_=b1t[s][:])

            xg_sb = {}
            w1_sb = {}
            w2_sb = {}
            for s, C in secs:
                xg_sb[s] = [
                    wpool.tile([P, C], bf16, name=f"xg_sb{s}{k}", tag=f"xg{s}{k}")
                    for k in range(K1)
                ]
                w1_sb[s] = [
                    wpool.tile([P, 2 * FH], bf16, name=f"w1_sb{s}{k}", tag=f"w1{s}{k}")
                    for k in range(K1)
                ]
                w2_sb[s] = [
                    wpool.tile([P, D_MODEL], bf16, name=f"w2_sb{s}{k}", tag=f"w2{s}{k}")
                    for k in range(K2)
                ]

            # PE warmup: dummy matmuls on a zeroed tile keep the PE busy (and
            # open the HAM clock gate to 2.4 GHz) until the first input DMAs
            # land (~8us fixed preamble + first chunks).
            warm = wpool.tile([P, NT], bf16, name="warm")
            nc.gpsimd.memset(warm[:], 0.0)
            ps_w = ps_pool.tile([P, NT], f32, name="ps_warm", tag="ps")
            for _ in range(28):
                nc.tensor.matmul(ps_w[:], warm[:, :P], warm[:], start=True, stop=True)

            # ---- bulk inputs on qSP in exact PE consumption order.
            # (qACT is unusable for inputs: dma_start issue on the ACT engine
            # stream blocks the PSUM-drain epilogue ACTs behind it; splitting
            # inputs across queues scrambles arrival order and stalls the PE.)
            def emit_xg(s, C, t, eng=None):
                eng = eng or nc.sync
                off, Nt = tile_bounds(C)[t]
                for k in range(K1):
                    eng.dma_start(
                        out=xg_sb[s][k][:, off:off + Nt],
                        in_=xT[s][k * P:(k + 1) * P, off:off + Nt],
                    )

            def emit_w1_chunk(s, c0, c1):
                for k in range(K1):
                    nc.sync.dma_start(
                        out=w1_sb[s][k][:, c0:c1],
                        in_=w1[s][k * P:(k + 1) * P, c0:c1],
                    )

            def emit_input_dmas(s, C, first=False):
                n_t = (C + NT - 1) // NT
                if first:
                    # supply matches the staggered compute start: tile 0's
                    # tokens ride qACT (8 early dma_starts are issued before
                    # any epilogue ACT exists) in parallel with w1 chunk 0 on
                    # qSP, then the other tiles follow on qSP
                    emit_xg(s, C, 0, eng=nc.scalar)
                    emit_w1_chunk(s, 0, WCHUNK)
                    for t in range(1, n_t):
                        emit_xg(s, C, t)
                    c0 = WCHUNK
                else:
                    for t in range(n_t):
                        emit_xg(s, C, t)
                    c0 = 0
                step = WCHUNK if first else 2 * WCHUNK
                bounds = list(range(c0, 2 * FH, step)) + [2 * FH]
                for b0, b1 in zip(bounds[:-1], bounds[1:]):
                    emit_w1_chunk(s, b0, b1)
                for k in range(K2):
                    nc.sync.dma_start(
                        out=w2_sb[s][k][:], in_=w2[s][k * P:(k + 1) * P, :],
                    )

            emit_input_dmas("a", CA, first=True)
            emit_input_dmas("b", CB)

            # ---- main loops ----
            # W1 columns host-permuted to [a_0 | g_0 | a_1 | g_1 | ...] so the
            # PE reads w1_sb left-to-right.  matmul1 iterates i-outer/t-inner:
            # all token tiles consume one weight block before moving on, so
            # the weight-DMA demand rate is ~halved and tokens (cheap, small)
            # are needed upfront instead of mid-stream.
            uT = {}

            def emit_mm1(s, C, staggered=False):
                n_t = (C + NT - 1) // NT
                for t in range(n_t):
                    uT[(s, t)] = upool.tile(
                        [P, K2, NT], bf16, name=f"uT{s}{t}", tag="uT", bufs=4)
                if staggered:
                    # first two i-blocks run tile 0 first so compute can start
                    # on just xa_t0 + w1 chunk 0 while the rest streams in
                    sched = [(0, 0), (1, 0)]
                    sched += [(i, t) for i in (0, 1) for t in range(1, n_t)]
                    sched += [(i, t) for i in range(2, K2) for t in range(n_t)]
                else:
                    sched = [(i, t) for i in range(K2) for t in range(n_t)]
                for i, t in sched:
                        off, Nt = tile_bounds(C)[t]
                        ps_a = ps_pool.tile(
                            [P, NT], f32, name=f"psa{s}{t}_{i}", tag="ps")
                        for k in range(K1):
                            nc.tensor.matmul(
                                ps_a[:, :Nt],
                                w1_sb[s][k][:, ts(2 * i, P)],
                                xg_sb[s][k][:, off:off + Nt],
                                start=(k == 0),
                                stop=(k == K1 - 1),
                            )
                        ps_g = ps_pool.tile(
                            [P, NT], f32, name=f"psg{s}{t}_{i}", tag="ps")
                        for k in range(K1):
                            nc.tensor.matmul(
                                ps_g[:, :Nt],
                                w1_sb[s][k][:, ts(2 * i + 1, P)],
                                xg_sb[s][k][:, off:off + Nt],
                                start=(k == 0),
                                stop=(k == K1 - 1),
                            )
                        a_t = epool.tile([P, NT], bf16, name=f"a{s}{t}_{i}", tag="a")
                        nc.scalar.activation(
                            a_t[:, :Nt], ps_a[:, :Nt], AF.Identity,
                            bias=b1_sb[s][:, 2 * i:2 * i + 1],
                        )
                        g_t = epool.tile([P, NT], bf16, name=f"g{s}{t}_{i}", tag="g")
                        if use_silu:
                            nc.scalar.activation(
                                g_t[:, :Nt], ps_g[:, :Nt], AF.Silu,
                                bias=b1_sb[s][:, 2 * i + 1:2 * i + 2],
                            )
                        else:
                            s_t = epool.tile(
                                [P, NT], bf16, name=f"s{s}{t}_{i}", tag="s")
                            nc.scalar.activation(
                                s_t[:, :Nt], ps_g[:, :Nt], AF.Sigmoid,
                                bias=b1_sb[s][:, 2 * i + 1:2 * i + 2],
                            )
                            gb_t = epool.tile(
                                [P, NT], bf16, name=f"gb{s}{t}_{i}", tag="gb")
                            nc.scalar.activation(
                                gb_t[:, :Nt], ps_g[:, :Nt], AF.Identity,
                                bias=b1_sb[s][:, 2 * i + 1:2 * i + 2],
                            )
                            nc.vector.tensor_mul(
                                g_t[:, :Nt], gb_t[:, :Nt], s_t[:, :Nt])
                        nc.vector.tensor_mul(
                            uT[(s, t)][:, i, :Nt], a_t[:, :Nt], g_t[:, :Nt])

            def emit_mm2(s, C):
                for t, (off, Nt) in enumerate(tile_bounds(C)):
                    for m in range(NO):
                        ps_y = ps_pool.tile(
                            [P, NT], f32, name=f"psy{s}{t}_{m}", tag="ps")
                        for k in range(K2):
                            nc.tensor.matmul(
                                ps_y[:, :Nt],
                                w2_sb[s][k][:, ts(m, P)],
                                uT[(s, t)][:, k, :Nt],
                                start=(k == 0),
                                stop=(k == K2 - 1),
                            )
                        # psum drain on DVE (idle), output via qSP behind the
                        # inputs: keeps the ScalarE stream free for the a/g
                        # drains (dma_start issue on ACT blocks them) and
                        # avoids SWDGE SBUF-read contention with the PE.
                        y_t = epool.tile([P, NT], bf16, name=f"y{s}{t}_{m}",
                                         tag="y", bufs=12)
                        nc.vector.tensor_copy(y_t[:, :Nt], ps_y[:, :Nt])
                        nc.sync.dma_start(
                            out=outp[s][m * P:(m + 1) * P, off:off + Nt],
                            in_=y_t[:, :Nt],
                        )

            emit_mm1("a", CA, staggered=True)
            emit_mm2("a", CA)
            emit_mm1("b", CB)
            emit_mm2("b", CB)

    nc.compile()
    return nc


def _route_tokens(xf, Wr, temp):
    """Bit-match the reference's router on CPU jax: logits, top-2, softmax."""
    import jax
    import jax.numpy as jnp

    cpu = jax.devices("cpu")[0]
    with jax.default_device(cpu):
        xj = jnp.asarray(xf)
        logits = (xj @ jnp.asarray(Wr)) / jnp.asarray(temp)
        topw, topi = jax.lax.top_k(logits, TOP_K)
        topw = jax.nn.softmax(topw, axis=-1)
    return np.asarray(topi), np.asarray(topw)


def _pad32(n):
    return max(P, ((n + 31) // 32) * 32)


def kernel(**inputs) -> np.ndarray:
    global LAST_RESULTS
    from concourse.bass_utils import run_bass_kernel_spmd

    x = np.asarray(inputs["x"], dtype=np.float32)
    Wr = np.asarray(inputs["Wr"], dtype=np.float32)
    temp = np.asarray(inputs["temp"], dtype=np.float32)
    W1 = np.asarray(inputs["W1"], dtype=np.float32)
    b1 = np.asarray(inputs["b1"], dtype=np.float32)
    W2 = np.asarray(inputs["W2"], dtype=np.float32)
    b2 = np.asarray(inputs["b2"], dtype=np.float32)

    B, S, D = x.shape
    T = B * S
    xf = x.reshape(T, D)

    topi, topw = _route_tokens(xf, Wr, temp)

    # Per-expert token lists and combine weights.
    tok_idx = []
    tok_w = []
    for e in range(NUM_EXPERTS):
        mask = topi == e                       # [T, K]
        sel = mask.any(axis=1)
        idx = np.nonzero(sel)[0]
        w = (topw * mask).sum(axis=1)[idx]
        tok_idx.append(idx)
        tok_w.append(w.astype(np.float32))

    counts = np.array([len(i) for i in tok_idx])
    order = np.argsort(-counts, kind="stable")
    bigs = list(order[:4])                     # section A experts
    smalls = list(order[4:])                   # section B experts
    CA = _pad32(max(counts[e] for e in bigs))
    CB = _pad32(max(counts[e] for e in smalls))

    # a/g interleave within a ff half: [a_0 | g_0 | a_1 | g_1 | ...]
    def w1_cols(h):
        cols = []
        for j in range(h * (FH // P), (h + 1) * (FH // P)):
            cols.append(np.arange(j * P, (j + 1) * P))            # a_j
            cols.append(np.arange(D_FF + j * P, D_FF + (j + 1) * P))  # g_j
        return np.concatenate(cols)

    cols_h = [w1_cols(0), w1_cols(1)]

    bf16 = ml_dtypes.bfloat16

    def xgT_of(e, C):
        idx = tok_idx[e]
        xg = np.zeros((C, D), dtype=np.float32)
        xg[: len(idx)] = xf[idx]
        return np.ascontiguousarray(xg.T).astype(bf16)

    xgT_cache = {e: xgT_of(e, CA if e in bigs else CB) for e in range(NUM_EXPERTS)}

    in_maps = []
    for c in range(N_CORES):
        h = c // 4
        m = {}
        for s, elist in (("a", bigs), ("b", smalls)):
            e = elist[c % 4]
            cols = cols_h[h]
            m[f"x{s}T"] = xgT_cache[e]
            m[f"w1{s}"] = np.ascontiguousarray(W1[e][:, cols]).astype(bf16)
            m[f"w2{s}"] = np.ascontiguousarray(
                W2[e][h * FH:(h + 1) * FH, :]).astype(bf16)
            m[f"b1t{s}"] = np.ascontiguousarray(
                b1[e][cols].reshape(2 * FH // P, P).T)
        in_maps.append(m)

    key = (CA, CB)
    if key not in _NC_CACHE:
        _NC_CACHE[key] = _build_nc(CA, CB)
    nc = _NC_CACHE[key]

    trace = bool(os.environ.get("MOE_KERNEL_TRACE"))
    kwargs = {}
    if trace:
        kwargs = dict(trace=True, trace_cores=list(range(N_CORES)))
    res = run_bass_kernel_spmd(nc, in_maps, core_ids=list(range(N_CORES)), **kwargs)
    LAST_RESULTS = res

    out = np.zeros((T, D), dtype=np.float32)
    for s, elist in (("a", bigs), ("b", smalls)):
        for i, e in enumerate(elist):
            idx = tok_idx[e]
            if len(idx) == 0:
                continue
            y0 = np.asarray(res.results[i]["out" + s]).astype(np.float32)
            y1 = np.asarray(res.results[i + 4]["out" + s]).astype(np.float32)
            y = (y0 + y1)[:, : len(idx)].T + b2[e]
            out[idx] += y * tok_w[e][:, None]

    return out.reshape(B, S, D)



# revision 4
# speedup vs baseline: 1.0206x; 1.0206x over previous
"""Trainium2 Bass kernel for AdaptiveMixtureOfExperts (top-2 SwiGLU MoE).

Strategy (expert-parallel, quarter-FF load balancing):
  - Host computes the tiny router (x @ Wr, top-2, softmax) with jax-on-CPU ops
    that bit-match the reference, then shards tokens by routed expert.
  - Each expert's FFN is split into 4 quarters along D_FF.  Experts are ranked
    by token count; rank pair (2p, 2p+1) forms section-position p, so each of
    the 8 cores runs 4 quarter-FFN sections (one per position).  Per-position
    token capacity = pad8(max count of the pair), giving per-core work within
    ~1.4% of the mean (vs ~4.7% for the old half-FF big/small pairing).
        hT = W1q.T @ xqT            (ff on partitions, tokens on free dim)
        uT = (a + b1a) * silu(g + b1g)
        yT_partial = W2q.T @ uT
  - Host sums the 4 quarter contributions per expert, adds b2, applies the
    top-2 combine weights, and scatter-adds into the full [B, S, D] output.

  DMA layout: all per-core inputs are host-packed into partition-major 3D
  blocks ([128, k, cols]) so each logical tensor moves in ONE fat DMA
  instruction (DMA issue costs ~0.6us of queue-engine time regardless of
  size).  All inputs ride qSP in exact consumption order; y outputs and the
  tiny b1 vectors ride qACT so neither queue ever head-of-line blocks the
  other's dependencies.

Shapes hardcoded for the problem instance:
  x:[2,2048,1024] f32, Wr:[1024,8], temp:[1], W1:[8,1024,4096], b1:[8,4096],
  W2:[8,2048,1024], b2:[8,1024].  TOP_K=2, 8 experts on 8 cores.
"""

import os

import numpy as np
import ml_dtypes

D_MODEL = 1024
D_FF = 2048
NUM_EXPERTS = 8
TOP_K = 2
P = 128          # partitions
NT = 512         # token tile (moving free dim per matmul)
T0 = 128         # first token tile of section 0 (small for an early start)
N_CORES = 8
NSEC = 4         # sections per core (one expert-quarter each)
FQ = D_FF // NSEC            # 512: ff quarter
K1 = D_MODEL // P            # 8 k-tiles for matmul1
K2 = FQ // P                 # 4 k-tiles for matmul2
NBLK = 2 * FQ // P           # 8 ff blocks per section (a/g interleaved)
NO = D_MODEL // P            # 8 output blocks of yT
WARMUP = 12                  # PE warmup matmuls (cover preamble+first DMA)

_NC_CACHE = {}
LAST_RESULTS = None  # test harness introspection


def _tiles(C, first=None):
    """Token tile (offset, size) list: optional small first tile, then NT."""
    cuts = [0]
    if first and first < C:
        cuts.append(first)
    while cuts[-1] < C:
        cuts.append(min(cuts[-1] + NT, C))
    return list(zip(cuts[:-1], (b - a for a, b in zip(cuts[:-1], cuts[1:]))))


def _build_nc(CS, use_silu: bool = True):
    """Per-core Bass graph: NSEC quarter-FF FFN sections of CS[s] tokens."""
    import concourse.mybir as mybir
    import concourse.tile as tile
    from concourse import bacc
    from concourse.bass import ts

    f32 = mybir.dt.float32
    bf16 = mybir.dt.bfloat16
    AF = mybir.ActivationFunctionType

    nc = bacc.Bacc()
    xq = {}
    w1 = {}
    w2 = {}
    b1t = {}
    outp = {}
    for s, C in enumerate(CS):
        xq[s] = nc.declare_dram_parameter(f"xq{s}", [P, K1, C], bf16, isOutput=False)
        w1[s] = nc.declare_dram_parameter(f"w1{s}", [P, K1, 2 * FQ], bf16, isOutput=False)
        w2[s] = nc.declare_dram_parameter(f"w2{s}", [P, K2, D_MODEL], bf16, isOutput=False)
        b1t[s] = nc.declare_dram_parameter(f"b1t{s}", [P, NBLK], f32, isOutput=False)
        # partial y without b2 (host adds the bias once per expert), bf16 to
        # halve output DMA bytes
        outp[s] = nc.declare_dram_parameter(f"out{s}", [P, NO, C], bf16, isOutput=True)

    CMAX = max(CS)

    with tile.TileContext(nc) as tc:
        with (
            tc.tile_pool(name="weights", bufs=1) as wpool,
            tc.tile_pool(name="acts", bufs=2) as upool,
            tc.tile_pool(name="epilogue", bufs=4) as epool,
            tc.tile_pool(name="ps", bufs=8, space="PSUM") as ps_pool,
        ):
            # ---- tiny early inputs on qACT (biases) ----
            b1_sb = {}
            for s in range(NSEC):
                b1_sb[s] = wpool.tile([P, NBLK], f32, name=f"b1_sb{s}", tag=f"b1{s}")
                nc.scalar.dma_start(out=b1_sb[s][:], in_=b1t[s][:])

            # weights resident per section; tokens double-buffered across
            # sections (section s+2's load waits for section s's reads)
            w1_sb = {}
            w2_sb = {}
            xg_sb = {}
            for s in range(NSEC):
                w1_sb[s] = wpool.tile([P, K1, 2 * FQ], bf16, name=f"w1_sb{s}",
                                      tag=f"w1{s}")
                w2_sb[s] = wpool.tile([P, K2, D_MODEL], bf16, name=f"w2_sb{s}",
                                      tag=f"w2{s}")
                xg_sb[s] = upool.tile([P, K1, CMAX], bf16, name=f"xg_sb{s}",
                                      tag="xg", bufs=2)

            # PE warmup: dummy matmuls on a zeroed tile keep the PE busy (and
            # open the HAM clock gate to 2.4 GHz) until the first input DMAs
            # land (~6us fixed preamble + first chunks).
            warm = wpool.tile([P, NT], bf16, name="warm")
            nc.gpsimd.memset(warm[:], 0.0)
            ps_w = ps_pool.tile([P, NT], f32, name="ps_warm", tag="ps")
            for _ in range(WARMUP):
                nc.tensor.matmul(ps_w[:], warm[:, :P], warm[:], start=True, stop=True)

            # ---- bulk inputs on qSP in exact PE consumption order ----
            def emit_w1(s, b0, b1_):
                nc.sync.dma_start(
                    out=w1_sb[s][:, :, b0:b1_], in_=w1[s][:, :, b0:b1_])

            def emit_xg(s, t0, t1):
                nc.sync.dma_start(
                    out=xg_sb[s][:, :, t0:t1], in_=xq[s][:, :, t0:t1])

            # section 0: minimal first chunk (w1 block 0 + first T0 tokens)
            # so real matmuls start as soon after the preamble as possible;
            # tokens arrive per-tile so i=0 can proceed tile-by-tile
            emit_w1(0, 0, 2 * P)
            for off, Nt in _tiles(CS[0], first=T0):
                emit_xg(0, off, off + Nt)
            emit_w1(0, 2 * P, 4 * P)
            emit_w1(0, 4 * P, 2 * FQ)
            nc.sync.dma_start(out=w2_sb[0][:], in_=w2[0][:])
            # section 1: w1 head, tokens, w1 tail, w2
            emit_w1(1, 0, 2 * P)
            emit_xg(1, 0, CS[1])
            emit_w1(1, 2 * P, 2 * FQ)
            nc.sync.dma_start(out=w2_sb[1][:], in_=w2[1][:])
            # sections 2/3: weights are wait-free; token loads reuse the xg
            # buffers of sections 0/1 and carry waits on their last reads --
            # ordered here so the wait is already satisfied (or harmless)
            # when the queue reaches it
            emit_w1(2, 0, 2 * FQ)
            nc.sync.dma_start(out=w2_sb[2][:], in_=w2[2][:])
            emit_xg(2, 0, CS[2])
            emit_w1(3, 0, 2 * FQ)
            emit_xg(3, 0, CS[3])
            nc.sync.dma_start(out=w2_sb[3][:], in_=w2[3][:])

            # ---- main loops ----
            # W1 columns host-permuted to [a_0 | g_0 | a_1 | g_1 | ...] so the
            # PE reads w1_sb left-to-right.  matmul1 iterates i-outer/t-inner.
            uT = {}

            def emit_mm1(s, staggered=False):
                tl = _tiles(CS[s], first=T0 if staggered else None)
                for t in range(len(tl)):
                    uT[(s, t)] = upool.tile(
                        [P, K2, NT], bf16, name=f"uT{s}{t}", tag="uT", bufs=4)
                for i in range(K2):
                    for t, (off, Nt) in enumerate(tl):
                        ps_a = ps_pool.tile(
                            [P, NT], f32, name=f"psa{s}{t}_{i}", tag="ps")
                        for k in range(K1):
                            nc.tensor.matmul(
                                ps_a[:, :Nt],
                                w1_sb[s][:, k, ts(2 * i, P)],
                                xg_sb[s][:, k, off:off + Nt],
                                start=(k == 0),
                                stop=(k == K1 - 1),
                            )
                        ps_g = ps_pool.tile(
                            [P, NT], f32, name=f"psg{s}{t}_{i}", tag="ps")
                        for k in range(K1):
                            nc.tensor.matmul(
                                ps_g[:, :Nt],
                                w1_sb[s][:, k, ts(2 * i + 1, P)],
                                xg_sb[s][:, k, off:off + Nt],
                                start=(k == 0),
                                stop=(k == K1 - 1),
                            )
                        a_t = epool.tile([P, NT], bf16, name=f"a{s}{t}_{i}",
                                         tag="a")
                        nc.scalar.activation(
                            a_t[:, :Nt], ps_a[:, :Nt], AF.Identity,
                            bias=b1_sb[s][:, 2 * i:2 * i + 1],
                        )
                        g_t = epool.tile([P, NT], bf16, name=f"g{s}{t}_{i}",
                                         tag="g")
                        if use_silu:
                            nc.scalar.activation(
                                g_t[:, :Nt], ps_g[:, :Nt], AF.Silu,
                                bias=b1_sb[s][:, 2 * i + 1:2 * i + 2],
                            )
                        else:
                            s_t = epool.tile(
                                [P, NT], bf16, name=f"s{s}{t}_{i}", tag="s")
                            nc.scalar.activation(
                                s_t[:, :Nt], ps_g[:, :Nt], AF.Sigmoid,
                                bias=b1_sb[s][:, 2 * i + 1:2 * i + 2],
                            )
                            gb_t = epool.tile(
                                [P, NT], bf16, name=f"gb{s}{t}_{i}", tag="gb")
                            nc.scalar.activation(
                                gb_t[:, :Nt], ps_g[:, :Nt], AF.Identity,
                                bias=b1_sb[s][:, 2 * i + 1:2 * i + 2],
                            )
                            nc.vector.tensor_mul(
                                g_t[:, :Nt], gb_t[:, :Nt], s_t[:, :Nt])
                        nc.vector.tensor_mul(
                            uT[(s, t)][:, i, :Nt], a_t[:, :Nt], g_t[:, :Nt])

            def emit_mm2(s, staggered=False, fine_tail=False):
                tl = _tiles(CS[s], first=T0 if staggered else None)
                for t, (off, Nt) in enumerate(tl):
                    last_tile = fine_tail and t == len(tl) - 1
                    y_t = epool.tile([P, NO, NT], bf16, name=f"y{s}{t}",
                                     tag="y", bufs=3)
                    for m in range(NO):
                        ps_y = ps_pool.tile(
                            [P, NT], f32, name=f"psy{s}{t}_{m}", tag="ps")
                        for k in range(K2):
                            nc.tensor.matmul(
                                ps_y[:, :Nt],
                                w2_sb[s][:, k, ts(m, P)],
                                uT[(s, t)][:, k, :Nt],
                                start=(k == 0),
                                stop=(k == K2 - 1),
                            )
                        # psum drain on DVE (idle), output via qACT (y DMAs +
                        # b1 are the only users, so the input stream on qSP is
                        # never blocked and y drains promptly)
                        nc.vector.tensor_copy(y_t[:, m, :Nt], ps_y[:, :Nt])
                        if last_tile:
                            nc.scalar.dma_start(
                                out=outp[s][:, m, off:off + Nt],
                                in_=y_t[:, m, :Nt],
                            )
                        elif m == NO // 2 - 1:
                            nc.scalar.dma_start(
                                out=outp[s][:, :NO // 2, off:off + Nt],
                                in_=y_t[:, :NO // 2, :Nt],
                            )
                        elif m == NO - 1:
                            nc.scalar.dma_start(
                                out=outp[s][:, NO // 2:, off:off + Nt],
                                in_=y_t[:, NO // 2:, :Nt],
                            )

            for s in range(NSEC):
                emit_mm1(s, staggered=(s == 0))
                emit_mm2(s, staggered=(s == 0), fine_tail=(s == NSEC - 1))

    nc.compile()
    return nc


def _route_tokens(xf, Wr, temp):
    """Bit-match the reference's router on CPU jax: logits, top-2, softmax."""
    import jax
    import jax.numpy as jnp

    cpu = jax.devices("cpu")[0]
    with jax.default_device(cpu):
        xj = jnp.asarray(xf)
        logits = (xj @ jnp.asarray(Wr)) / jnp.asarray(temp)
        topw, topi = jax.lax.top_k(logits, TOP_K)
        topw = jax.nn.softmax(topw, axis=-1)
    return np.asarray(topi), np.asarray(topw)


def _pad8(n):
    return max(P, ((n + 7) // 8) * 8)


def kernel(**inputs) -> np.ndarray:
    global LAST_RESULTS
    from concourse.bass_utils import run_bass_kernel_spmd

    x = np.asarray(inputs["x"], dtype=np.float32)
    Wr = np.asarray(inputs["Wr"], dtype=np.float32)
    temp = np.asarray(inputs["temp"], dtype=np.float32)
    W1 = np.asarray(inputs["W1"], dtype=np.float32)
    b1 = np.asarray(inputs["b1"], dtype=np.float32)
    W2 = np.asarray(inputs["W2"], dtype=np.float32)
    b2 = np.asarray(inputs["b2"], dtype=np.float32)

    B, S, D = x.shape
    T = B * S
    xf = x.reshape(T, D)

    topi, topw = _route_tokens(xf, Wr, temp)

    # Per-expert token lists and combine weights.
    tok_idx = []
    tok_w = []
    for e in range(NUM_EXPERTS):
        mask = topi == e                       # [T, K]
        sel = mask.any(axis=1)
        idx = np.nonzero(sel)[0]
        w = (topw * mask).sum(axis=1)[idx]
        tok_idx.append(idx)
        tok_w.append(w.astype(np.float32))

    counts = np.array([len(i) for i in tok_idx])
    order = np.argsort(-counts, kind="stable")
    # position p holds quarters of experts ranked 2p and 2p+1; core c runs
    # quarter c%4 of expert rank 2p + c//4 at position p
    pos_exp = [(order[2 * p], order[2 * p + 1]) for p in range(NSEC)]
    CS = tuple(_pad8(max(counts[ea], counts[eb])) for ea, eb in pos_exp)

    bf16 = ml_dtypes.bfloat16

    # a/g interleave within an ff quarter: [a_0 | g_0 | a_1 | g_1 | ...]
    def w1_cols(q):
        cols = []
        for j in range(q * K2, (q + 1) * K2):
            cols.append(np.arange(j * P, (j + 1) * P))                # a_j
            cols.append(np.arange(D_FF + j * P, D_FF + (j + 1) * P))  # g_j
        return np.concatenate(cols)

    cols_q = [w1_cols(q) for q in range(NSEC)]

    def xqT_of(e, C):
        idx = tok_idx[e]
        xg = np.zeros((C, D), dtype=np.float32)
        xg[: len(idx)] = xf[idx]
        # [C, (K1 P)] -> [P, K1, C] partition-major for one fat DMA
        out = np.ascontiguousarray(
            xg.reshape(C, K1, P).transpose(2, 1, 0)).astype(bf16)
        return out

    xqT_cache = {}
    for p, (ea, eb) in enumerate(pos_exp):
        for e in (ea, eb):
            xqT_cache[e] = xqT_of(e, CS[p])

    w1q = {}
    w2q = {}
    b1q = {}
    for e in range(NUM_EXPERTS):
        for q in range(NSEC):
            cols = cols_q[q]
            # W1[e][:, cols]: [(K1 P), 2FQ] -> [P, K1, 2FQ]
            w1q[(e, q)] = np.ascontiguousarray(
                W1[e][:, cols].reshape(K1, P, 2 * FQ).transpose(1, 0, 2)
            ).astype(bf16)
            # W2[e][q*FQ:(q+1)*FQ, :]: [(K2 P), D] -> [P, K2, D]
            w2q[(e, q)] = np.ascontiguousarray(
                W2[e][q * FQ:(q + 1) * FQ, :].reshape(K2, P, D_MODEL)
                .transpose(1, 0, 2)
            ).astype(bf16)
            b1q[(e, q)] = np.ascontiguousarray(
                b1[e][cols].reshape(NBLK, P).T)

    in_maps = []
    for c in range(N_CORES):
        q = c % 4
        m = {}
        for p, (ea, eb) in enumerate(pos_exp):
            e = ea if c < 4 else eb
            m[f"xq{p}"] = xqT_cache[e]
            m[f"w1{p}"] = w1q[(e, q)]
            m[f"w2{p}"] = w2q[(e, q)]
            m[f"b1t{p}"] = b1q[(e, q)]
        in_maps.append(m)

    if CS not in _NC_CACHE:
        _NC_CACHE[CS] = _build_nc(CS)
    nc = _NC_CACHE[CS]

    trace = bool(os.environ.get("MOE_KERNEL_TRACE"))
    kwargs = {}
    if trace:
        kwargs = dict(trace=True, trace_cores=list(range(N_CORES)))
    res = run_bass_kernel_spmd(nc, in_maps, core_ids=list(range(N_CORES)), **kwargs)
    LAST_RESULTS = res

    out = np.zeros((T, D), dtype=np.float32)
    for p, (ea, eb) in enumerate(pos_exp):
        for h, e in enumerate((ea, eb)):
            idx = tok_idx[e]
            if len(idx) == 0:
                continue
            # sum the 4 quarter partials (cores 4h..4h+3, section p)
            y = None
            for q in range(NSEC):
                yq = np.asarray(
                    res.results[4 * h + q][f"out{p}"]).astype(np.float32)
                y = yq if y is None else y + yq
            # [P, NO, C] -> [C, (NO P)]
            y = y.transpose(2, 1, 0).reshape(CS[p], D)[: len(idx)] + b2[e]
            out[idx] += y * tok_w[e][:, None]

    return out.reshape(B, S, D)
